# revision 1
# baseline (speedup 1.0000x reference)
"""Self-contained TRN2 Bass kernel for the decoder block (kernel.py shim).

During development this imports the dk module next to it. The final
submitted kernel.py will inline dk's contents.
"""

import numpy as np

from concourse.bass_utils import run_bass_kernel_spmd

import dk

LAST_RESULTS = {}
_CACHE = {}


def kernel(
    x, ln1_g, ln1_b, w_qkv, w_proj, ln2_g, ln2_b, w_fc1, w_fc2, _trace=False
):
    cfg = dk.Cfg()
    in_maps, assemble = dk.host_prep(
        cfg, x, ln1_g, ln1_b, w_qkv, w_proj, ln2_g, ln2_b, w_fc1, w_fc2
    )
    if "nc" not in _CACHE:
        _CACHE["nc"] = dk.build(cfg)
    nc = _CACHE["nc"]
    res = run_bass_kernel_spmd(
        nc, in_maps, core_ids=list(range(8)), trace=_trace
    )
    LAST_RESULTS["res"] = res
    return assemble(res.results)
